# revision 4
# baseline (speedup 1.0000x reference)
"""Trainium2 Bass kernel for nn_Eq1to2 (segment_reduce / equivariant 1->2 layer).

Math (derived from the reference):
  out[n,i,j,s] = leaky_relu( A[n,i,s] + B[n,j,s] + G[n,s]
                             + (i==j) * (D[n,i,s] + Gd[n,s]) ) * mask
with
  A  = x @ W3                       (col term, i-dependent)
  B  = x @ W2                       (row term, j-dependent)
  D  = x @ W1                       (extra diagonal term)
  G  = sum_a agg_a @ W5_a + bias    (per-sample constant)
  Gd = sum_a agg_a @ W4_a           (per-sample diagonal constant)
where the 20 basis ops collapse to W1..W3 = sums of 4 coef slices each and
per-aggregation W4_a / W5_a; agg_a in {sum/49, sum/nobj, max, min} over N.

Sharding: pure data parallel, 1 batch sample per NeuronCore (B=8, 8 cores).

Device strategy per core (output tile [i=128 part, (j,s)=8192 free] fp32):
  - the fp16 rhs for any 512-col chunk is the SAME [64, 512] tile
    (tile(W3, 8)) because rhs[c, j*64+s] = W3[c, s] depends only on
    s = col mod 64 -> a single 64 KB W3 tile replaces the 1 MB
    replicated-rhs broadcast of the previous design.
  - per 512-col chunk: mm1 = xT @ W3tile (K=64, start) then mm2 = K=1
    accumulate of the flat B+G+bias row (ones_p0 lhsT) -> PSUM has
    A + B + G + bias.
  - G / Gd are accumulated straight into the B'/diag PSUM via 3 extra
    matmuls with a stride-0-broadcast agg column as lhsT (no psum_gg ->
    cast -> K=1 hop chain).
  - eviction fuses leaky_relu: ACT Lrelu(alpha=.01) PSUM->SBUF on cols
    0:512 of each 1024 chunk, DVE scalar_tensor_tensor on cols 512:1024,
    in parallel.
  - bulk output DMAs all ride the Sync HWDGE ring as clean 4 KB packets;
    the i==j diagonal rows are overwritten with the exact-fp32 corrected
    values by small DMAs on the GpSimd SWDGE ring, ordered after the
    corresponding bulk via the framework's DRAM dependency tracking. The
    last chunk's 16 diag-carrying rows are written first (small DMA) so
    the final diag patch hides under the last big bulk.
"""

import numpy as np

B, N, C, S = 8, 128, 64, 64
AVG_NOBJ = np.float32(49.0)
NEG = 0.01

# fp16 packed input column layout (single input tensor [128, _BNF])
_BLH = 0         # lhsT [65, 128]: rows 0:64 xT, row 64 ones
_ONES0 = 128     # ones row at partition 0 [1, 128] (K=1 matmul lhsT)
_W2B = 256       # [65, 64]: rows W2, row 64 bias
_W1CB = 320      # [65, 64]: rows W1+W2+W3, row 64 bias
_G0 = 384        # 3 x [64, 64] G rhs blocks (W5sm, W5max, W5min)
_GD0 = 576       # 3 x [64, 64] Gd rhs blocks (W4+W5 combined per agg)
_W3T = 768       # W3 tiled x8 [64, 512]
_BNF = 1280
_NC1 = 256       # first (critical) input DMA: lhsT + ones

_CACHE = {}


def _build_nc():
    import concourse.bacc as bacc
    import concourse.bass as bass
    import concourse.mybir as mybir
    from concourse.tile import TileContext

    F32 = mybir.dt.float32
    FP16 = mybir.dt.float16
    Alu = mybir.AluOpType
    Act = mybir.ActivationFunctionType

    nc = bacc.Bacc("TRN2", debug=False, num_devices=8)
    inpb_d = nc.dram_tensor("inpb", [128, _BNF], FP16, kind="ExternalInput")
    out_d = nc.dram_tensor("out", [128, N * S], F32, kind="ExternalOutput")

    NB = 8           # chunks
    CW = 1024        # chunk width (free elems)

    with TileContext(nc) as tc:
        with tc.tile_pool(name="main", bufs=1) as pool, \
             tc.tile_pool(name="pz", bufs=3, space="PSUM") as pzpool, \
             tc.tile_pool(name="psm", bufs=1, space="PSUM") as psmpool:

            inpb = pool.tile([128, _BNF], FP16)
            bgflat = pool.tile([1, N * S], FP16)
            outbuf = pool.tile([128, N * S], F32)
            aggs = pool.tile([64, 4], FP16)
            bp_hi = pool.tile([128, 64], FP16)
            dleaky = pool.tile([128, 64], F32)

            # critical input first (xT + ones), then the weight blocks on
            # the other HWDGE ring
            nc.sync.dma_start(out=inpb[:, 0:_NC1], in_=inpb_d[:, 0:_NC1])
            nc.scalar.dma_start(out=inpb[:, _NC1:_BNF],
                                in_=inpb_d[:, _NC1:_BNF])

            lhsT = inpb[0:65, _BLH:_BLH + 128]
            ones_p0 = inpb[0:1, _ONES0:_ONES0 + 128]
            xT = inpb[0:64, _BLH:_BLH + 128]
            w3t = inpb[0:64, _W3T:_W3T + 512]

            # aggregations over N (free dim of xT)
            with nc.allow_low_precision("DVE reduces in fp32; fp16 is only "
                                        "the final rounding of the agg vec"):
                nc.vector.tensor_reduce(out=aggs[:, 0:1], in_=xT,
                                        axis=mybir.AxisListType.X, op=Alu.add)
            nc.vector.tensor_reduce(out=aggs[:, 1:2], in_=xT,
                                    axis=mybir.AxisListType.X, op=Alu.max)
            nc.vector.tensor_reduce(out=aggs[:, 2:3], in_=xT,
                                    axis=mybir.AxisListType.X, op=Alu.min)

            # psum_sm: cols 0:64 = diag z, cols 64:128 = B' + G + bias
            psum_sm = psmpool.tile([128, 128], F32)
            psum_diag = psum_sm[:, 0:64]
            psum_bp = psum_sm[:, 64:128]

            # B' = x @ W2 + bias, then += G_a via agg-broadcast lhsT matmuls
            nc.tensor.matmul(psum_bp, lhsT, inpb[0:65, _W2B:_W2B + 64],
                             start=True, stop=False)
            for a in range(3):
                nc.tensor.matmul(psum_bp,
                                 aggs[:, a:a + 1].broadcast_to([64, 128]),
                                 inpb[0:64, _G0 + 64 * a:_G0 + 64 * (a + 1)],
                                 start=False, stop=(a == 2),
                                 skip_group_check=True)
            # diag z = x @ (W1+W2+W3) + bias + sum_a agg_a @ (W4_a + W5_a)
            nc.tensor.matmul(psum_diag, lhsT, inpb[0:65, _W1CB:_W1CB + 64],
                             start=True, stop=False)
            for a in range(3):
                nc.tensor.matmul(psum_diag,
                                 aggs[:, a:a + 1].broadcast_to([64, 128]),
                                 inpb[0:64, _GD0 + 64 * a:_GD0 + 64 * (a + 1)],
                                 start=False, stop=(a == 2),
                                 skip_group_check=True)

            # flat BG row for the K=1 accumulate matmuls (partition gather)
            nc.scalar.copy(bp_hi[:, :], psum_bp)
            nc.sync.dma_start(out=bgflat[0:1, :], in_=bp_hi[:, :])

            # diag path (exact fp32): leaky(A+B+D+G+Gd+bias)
            nc.scalar.activation(dleaky[:, :], psum_diag, Act.Lrelu,
                                 alpha=NEG)

            flat = out_d[:, :].rearrange("a b -> (a b)")
            for c in range(NB):
                pz = pzpool.tile([128, CW], F32)
                for h in range(2):
                    o = pz[:, h * 512:(h + 1) * 512]
                    nc.tensor.matmul(o, xT, w3t, start=True, stop=False)
                    nc.tensor.matmul(o, ones_p0,
                                     bgflat[0:1, c * CW + h * 512:
                                            c * CW + (h + 1) * 512],
                                     start=False, stop=True)
                sl = slice(c * CW, (c + 1) * CW)
                # fused eviction + leaky straight from PSUM (DVE cannot
                # read two non-scalar PSUM operands, so ACT takes it all)
                nc.scalar.activation(outbuf[:, sl], pz[:, :], Act.Lrelu,
                                     alpha=NEG)
                if c < NB - 1:
                    nc.sync.dma_start(out=out_d[:, sl], in_=outbuf[:, sl])
                else:
                    # last chunk: its 16 diag-carrying rows first, so the
                    # final diag patch can land while the big bulk drains
                    nc.sync.dma_start(out=out_d[112:128, sl],
                                      in_=outbuf[112:128, sl])
                    nc.sync.dma_start(out=out_d[0:112, sl],
                                      in_=outbuf[0:112, sl])
                # diag rows i in [16c, 16c+16): overwrite out[i, i*64:+64]
                # with the exact values; ordered after the bulk by the
                # framework's DRAM dependency tracking (different queue)
                r0 = 16 * c
                dap = bass.AP(flat.tensor, flat.offset + r0 * (N + 1) * S,
                              [[(N + 1) * S, 16], [1, S]])
                nc.gpsimd.dma_start(out=dap, in_=dleaky[r0:r0 + 16, :])

    nc.compile()
    return nc


def _get_nc():
    if "nc" not in _CACHE:
        _CACHE["nc"] = _build_nc()
    return _CACHE["nc"]


def _host_pack(inputs, nobj, coefs, bias):
    x = np.asarray(inputs, np.float32)        # [B, N, C]
    nobj = np.asarray(nobj, np.float32)       # [B]
    c = np.asarray(coefs, np.float32)         # [C, S, 20]
    bias = np.asarray(bias, np.float32)       # [S]

    W1 = c[:, :, 0] + c[:, :, 5] + c[:, :, 10] + c[:, :, 15]
    W2 = c[:, :, 1] + c[:, :, 6] + c[:, :, 11] + c[:, :, 16]
    W3 = c[:, :, 2] + c[:, :, 7] + c[:, :, 12] + c[:, :, 17]
    W4 = [c[:, :, 3 + 5 * a] for a in range(4)]   # sum, mean, max, min
    W5 = [c[:, :, 4 + 5 * a] for a in range(4)]

    f16 = np.float16
    W3_t = np.tile(W3.astype(f16), (1, 8))

    in_maps = []
    for n in range(B):
        inpb = np.zeros((128, _BNF), f16)
        inpb[0:64, _BLH:_BLH + 128] = x[n].T.astype(f16)
        inpb[64, _BLH:_BLH + 128] = 1.0
        inpb[0, _ONES0:_ONES0 + 128] = 1.0
        inpb[0:64, _W2B:_W2B + 64] = W2.astype(f16)
        inpb[64, _W2B:_W2B + 64] = bias.astype(f16)
        inpb[0:64, _W1CB:_W1CB + 64] = (W1 + W2 + W3).astype(f16)
        inpb[64, _W1CB:_W1CB + 64] = bias.astype(f16)
        W4sm = W4[0] / AVG_NOBJ + W4[1] / nobj[n]
        W5sm = W5[0] / AVG_NOBJ + W5[1] / nobj[n]
        gs = [W5sm, W5[2], W5[3]]
        gds = [W4sm + W5sm, W4[2] + W5[2], W4[3] + W5[3]]
        for a in range(3):
            inpb[0:64, _G0 + 64 * a:_G0 + 64 * (a + 1)] = gs[a].astype(f16)
            inpb[0:64, _GD0 + 64 * a:_GD0 + 64 * (a + 1)] = gds[a].astype(f16)
        inpb[0:64, _W3T:_W3T + 512] = W3_t
        in_maps.append({"inpb": inpb})
    return in_maps


def _run(inputs, mask, nobj, coefs, bias, trace=False, **trace_kwargs):
    from concourse.bass_utils import run_bass_kernel_spmd

    in_maps = _host_pack(inputs, nobj, coefs, bias)
    nc = _get_nc()
    res = run_bass_kernel_spmd(nc, in_maps, list(range(B)), trace=trace,
                               **trace_kwargs)
    out = np.stack([res.results[i]["out"].reshape(N, N, S) for i in range(B)])
    m = np.asarray(mask, np.float32)
    if not np.all(m == 1.0):
        out = out * m  # mask is ones in the reference setup; host fallback
    return out, res


def kernel(inputs, mask, nobj, coefs, bias):
    out, _ = _run(inputs, mask, nobj, coefs, bias, trace=False)
    return out


if __name__ == "__main__":
    rng = np.random.default_rng(0)
    inputs = rng.standard_normal((B, N, C)).astype(np.float32)
    mask = np.ones((B, N, N, 1), np.float32)
    nobj = np.full((B,), 100.0, np.float32)
    coefs = (rng.standard_normal((C, S, 20)) * np.sqrt(2.0 / (C * 20))).astype(np.float32)
    bias = np.zeros((S,), np.float32)
    out = kernel(inputs, mask, nobj, coefs, bias)
    print("out", out.shape, out.dtype, float(np.abs(out).max()))


# revision 5
# speedup vs baseline: 1.1086x; 1.1086x over previous
"""Trainium2 Bass kernel for nn_Eq1to2 (segment_reduce / equivariant 1->2 layer).

Math (derived from the reference):
  out[n,i,j,s] = leaky_relu( A[n,i,s] + B[n,j,s] + G[n,s]
                             + (i==j) * (D[n,i,s] + Gd[n,s]) ) * mask
with
  A  = x @ W3                       (col term, i-dependent)
  B  = x @ W2                       (row term, j-dependent)
  D  = x @ W1                       (extra diagonal term)
  G  = sum_a agg_a @ W5_a + bias    (per-sample constant)
  Gd = sum_a agg_a @ W4_a           (per-sample diagonal constant)
where the 20 basis ops collapse to W1..W3 = sums of 4 coef slices each and
per-aggregation W4_a / W5_a; agg_a in {sum/49, sum/nobj, max, min} over N.

Sharding: pure data parallel, 1 batch sample per NeuronCore (B=8, 8 cores).

Device strategy per core (output tile [i=128 part, (j,s)=8192 free] fp32):
  - ONE fp16 K=65 matmul per 512-col chunk: lhsT=[xT; ones],
    rhs=[W3 tiled x128 (host-pretiled, loaded straight from DRAM);
    flat(B + G + bias) row written in place by a partition-gather DMA]
    -> PSUM holds A + B + G + bias (empirically ~604ns per 512-col mm;
    K=1 accumulate variants cost the same per column, so fusing
    everything into one K=65 pass minimizes PE time).
  - G / Gd accumulate straight into the B'/diag PSUM via 3 matmuls with
    a stride-0-broadcast agg column as lhsT (no psum_gg->cast->K=1 hops).
  - eviction fuses leaky_relu: one ACT Lrelu(alpha=.01) PSUM->SBUF per
    1024-col chunk (ACT only ever runs Lrelu -> a single table load,
    hidden early; bp_hi's fp16 cast runs on DVE instead).
  - all bulk output DMAs ride the Sync HWDGE ring as clean 4 KB packets;
    the i==j diagonal rows are overwritten with exact-fp32 corrected
    values by TWO small DMAs on the GpSimd SWDGE ring, ordered after the
    bulks via tracked DRAM deps. The last chunk's 16 diag-carrying rows
    go out as a separate small bulk first, so the final diag patch hides
    under the last big bulk.
"""

import numpy as np

B, N, C, S = 8, 128, 64, 64
AVG_NOBJ = np.float32(49.0)
NEG = 0.01

# fp16 packed input column layout (single input tensor [128, _BNF])
_BLH = 0         # lhsT [65, 128]: rows 0:64 xT, row 64 ones
_W2B = 128       # [65, 64]: rows W2, row 64 bias
_W1CB = 192      # [65, 64]: rows W1+W2+W3, row 64 bias
_G0 = 256        # 3 x [64, 64] G rhs blocks (W5sm, W5max, W5min)
_GD0 = 448       # 3 x [64, 64] Gd rhs blocks (W4+W5 combined per agg)
_W3R = 640       # W3 tiled x128 [64, 8192]; row 64 = BG row (device-built)
_BNF = 640 + 8192
_NC1 = 128       # first (critical) input DMA: lhsT
_NC2 = 640       # second: small weight blocks

_CACHE = {}


def _build_nc():
    import concourse.bacc as bacc
    import concourse.bass as bass
    import concourse.mybir as mybir
    from concourse.tile import TileContext

    F32 = mybir.dt.float32
    FP16 = mybir.dt.float16
    Alu = mybir.AluOpType
    Act = mybir.ActivationFunctionType

    nc = bacc.Bacc("TRN2", debug=False, num_devices=8)
    inpb_d = nc.dram_tensor("inpb", [128, _BNF], FP16, kind="ExternalInput")
    out_d = nc.dram_tensor("out", [128, N * S], F32, kind="ExternalOutput")

    NB = 8           # chunks
    CW = 1024        # chunk width (free elems)

    with TileContext(nc) as tc:
        with tc.tile_pool(name="main", bufs=1) as pool, \
             tc.tile_pool(name="pz", bufs=3, space="PSUM") as pzpool, \
             tc.tile_pool(name="psm", bufs=1, space="PSUM") as psmpool:

            inpb = pool.tile([128, _BNF], FP16)
            outbuf = pool.tile([128, N * S], F32)
            aggs = pool.tile([64, 4], FP16)
            bp_hi = pool.tile([128, 64], FP16)
            dleaky = pool.tile([128, 64], F32)

            # critical input first (xT), small weight blocks + the 1MB
            # pretiled W3 block (partitions 0:64 only) on the other ring
            nc.sync.dma_start(out=inpb[:, 0:_NC1], in_=inpb_d[:, 0:_NC1])
            nc.scalar.dma_start(out=inpb[:, _NC1:_NC2],
                                in_=inpb_d[:, _NC1:_NC2])
            nc.scalar.dma_start(out=inpb[0:64, _NC2:_BNF],
                                in_=inpb_d[0:64, _NC2:_BNF])

            lhsT = inpb[0:65, _BLH:_BLH + 128]
            xT = inpb[0:64, _BLH:_BLH + 128]

            # aggregations over N (free dim of xT)
            with nc.allow_low_precision("DVE reduces in fp32; fp16 is only "
                                        "the final rounding of the agg vec"):
                nc.vector.tensor_reduce(out=aggs[:, 0:1], in_=xT,
                                        axis=mybir.AxisListType.X, op=Alu.add)
            nc.vector.tensor_reduce(out=aggs[:, 1:2], in_=xT,
                                    axis=mybir.AxisListType.X, op=Alu.max)
            nc.vector.tensor_reduce(out=aggs[:, 2:3], in_=xT,
                                    axis=mybir.AxisListType.X, op=Alu.min)

            # psum_sm: cols 0:64 = diag z, cols 64:128 = B' + G + bias
            psum_sm = psmpool.tile([128, 128], F32)
            psum_diag = psum_sm[:, 0:64]
            psum_bp = psum_sm[:, 64:128]

            # B' = x @ W2 + bias, then += G_a via agg-broadcast lhsT matmuls
            nc.tensor.matmul(psum_bp, lhsT, inpb[0:65, _W2B:_W2B + 64],
                             start=True, stop=False)
            for a in range(3):
                nc.tensor.matmul(psum_bp,
                                 aggs[:, a:a + 1].broadcast_to([64, 128]),
                                 inpb[0:64, _G0 + 64 * a:_G0 + 64 * (a + 1)],
                                 start=False, stop=(a == 2),
                                 skip_group_check=True)
            # diag z = x @ (W1+W2+W3) + bias + sum_a agg_a @ (W4_a + W5_a)
            nc.tensor.matmul(psum_diag, lhsT, inpb[0:65, _W1CB:_W1CB + 64],
                             start=True, stop=False)
            for a in range(3):
                nc.tensor.matmul(psum_diag,
                                 aggs[:, a:a + 1].broadcast_to([64, 128]),
                                 inpb[0:64, _GD0 + 64 * a:_GD0 + 64 * (a + 1)],
                                 start=False, stop=(a == 2),
                                 skip_group_check=True)

            # BG row: fp16 cast on DVE (keeps ACT Lrelu-only -> one table
            # load), then partition-gather it into rhs row 64
            nc.vector.tensor_copy(bp_hi[:, :], psum_bp)
            nc.sync.dma_start(out=inpb[64:65, _NC2:_BNF], in_=bp_hi[:, :])

            # diag path (exact fp32): leaky(A+B+D+G+Gd+bias); also the
            # first ACT op -> Lrelu table loads early, off the chunk path
            nc.scalar.activation(dleaky[:, :], psum_diag, Act.Lrelu,
                                 alpha=NEG)

            flat = out_d[:, :].rearrange("a b -> (a b)")

            def diag_patch(r0, r1):
                dap = bass.AP(flat.tensor, flat.offset + r0 * (N + 1) * S,
                              [[(N + 1) * S, r1 - r0], [1, S]])
                nc.gpsimd.dma_start(out=dap, in_=dleaky[r0:r1, :])

            for c in range(NB):
                pz = pzpool.tile([128, CW], F32)
                for h in range(2):
                    o = pz[:, h * 512:(h + 1) * 512]
                    r = inpb[0:65, _NC2 + c * CW + h * 512:
                             _NC2 + c * CW + (h + 1) * 512]
                    nc.tensor.matmul(o, lhsT, r, start=True, stop=True)
                sl = slice(c * CW, (c + 1) * CW)
                # fused eviction + leaky straight from PSUM
                nc.scalar.activation(outbuf[:, sl], pz[:, :], Act.Lrelu,
                                     alpha=NEG)
                if c < NB - 1:
                    nc.sync.dma_start(out=out_d[:, sl], in_=outbuf[:, sl])
                else:
                    # last chunk: its 16 diag-carrying rows first, so the
                    # final diag patch lands while the big bulk drains
                    nc.sync.dma_start(out=out_d[112:128, sl],
                                      in_=outbuf[112:128, sl])
                    diag_patch(112, 128)
                    nc.sync.dma_start(out=out_d[0:112, sl],
                                      in_=outbuf[0:112, sl])
                if c == NB - 2:
                    # diag rows 0:112 live in chunks 0..6; one patch DMA
                    # ordered after those bulks by tracked DRAM deps
                    diag_patch(0, 112)

    nc.compile()
    return nc


def _get_nc():
    if "nc" not in _CACHE:
        _CACHE["nc"] = _build_nc()
    return _CACHE["nc"]


def _host_pack(inputs, nobj, coefs, bias):
    x = np.asarray(inputs, np.float32)        # [B, N, C]
    nobj = np.asarray(nobj, np.float32)       # [B]
    c = np.asarray(coefs, np.float32)         # [C, S, 20]
    bias = np.asarray(bias, np.float32)       # [S]

    W1 = c[:, :, 0] + c[:, :, 5] + c[:, :, 10] + c[:, :, 15]
    W2 = c[:, :, 1] + c[:, :, 6] + c[:, :, 11] + c[:, :, 16]
    W3 = c[:, :, 2] + c[:, :, 7] + c[:, :, 12] + c[:, :, 17]
    W4 = [c[:, :, 3 + 5 * a] for a in range(4)]   # sum, mean, max, min
    W5 = [c[:, :, 4 + 5 * a] for a in range(4)]

    f16 = np.float16
    W3_t = np.tile(W3.astype(f16), (1, 128))

    in_maps = []
    for n in range(B):
        inpb = np.zeros((128, _BNF), f16)
        inpb[0:64, _BLH:_BLH + 128] = x[n].T.astype(f16)
        inpb[64, _BLH:_BLH + 128] = 1.0
        inpb[0:64, _W2B:_W2B + 64] = W2.astype(f16)
        inpb[64, _W2B:_W2B + 64] = bias.astype(f16)
        inpb[0:64, _W1CB:_W1CB + 64] = (W1 + W2 + W3).astype(f16)
        inpb[64, _W1CB:_W1CB + 64] = bias.astype(f16)
        W4sm = W4[0] / AVG_NOBJ + W4[1] / nobj[n]
        W5sm = W5[0] / AVG_NOBJ + W5[1] / nobj[n]
        gs = [W5sm, W5[2], W5[3]]
        gds = [W4sm + W5sm, W4[2] + W5[2], W4[3] + W5[3]]
        for a in range(3):
            inpb[0:64, _G0 + 64 * a:_G0 + 64 * (a + 1)] = gs[a].astype(f16)
            inpb[0:64, _GD0 + 64 * a:_GD0 + 64 * (a + 1)] = gds[a].astype(f16)
        inpb[0:64, _W3R:_BNF] = W3_t
        in_maps.append({"inpb": inpb})
    return in_maps


def _run(inputs, mask, nobj, coefs, bias, trace=False, **trace_kwargs):
    from concourse.bass_utils import run_bass_kernel_spmd

    in_maps = _host_pack(inputs, nobj, coefs, bias)
    nc = _get_nc()
    res = run_bass_kernel_spmd(nc, in_maps, list(range(B)), trace=trace,
                               **trace_kwargs)
    out = np.stack([res.results[i]["out"].reshape(N, N, S) for i in range(B)])
    m = np.asarray(mask, np.float32)
    if not np.all(m == 1.0):
        out = out * m  # mask is ones in the reference setup; host fallback
    return out, res


def kernel(inputs, mask, nobj, coefs, bias):
    out, _ = _run(inputs, mask, nobj, coefs, bias, trace=False)
    return out


if __name__ == "__main__":
    rng = np.random.default_rng(0)
    inputs = rng.standard_normal((B, N, C)).astype(np.float32)
    mask = np.ones((B, N, N, 1), np.float32)
    nobj = np.full((B,), 100.0, np.float32)
    coefs = (rng.standard_normal((C, S, 20)) * np.sqrt(2.0 / (C * 20))).astype(np.float32)
    bias = np.zeros((S,), np.float32)
    out = kernel(inputs, mask, nobj, coefs, bias)
    print("out", out.shape, out.dtype, float(np.abs(out).max()))
